# revision 8
# baseline (speedup 1.0000x reference)
"""Trainium2 Bass kernel for nn_FCPairedLayer (gnn_message_passing).

Reference computation:
    v[b,i,j] = concat(x_i, x_j, x_{i-1}*m1, x_{j+1}*m1, x_{i+1}*m2, x_{j-1}*m2)
    y[b,i,j] = W2 @ relu(W1 @ v + b1) + b2        (scalar output per pair)
with m1 = [i>=1][j<=N-2], m2 = [i<=N-2][j>=1].

W1 @ v splits into row-only and column-only terms; per batch define
    R[:,i] = W1_a x_i + W1_c x_{i-1} + W1_e x_{i+1} + b1     (shifts masked)
    C[:,j] = W1_b x_j + W1_d x_{j+1} + W1_f x_{j-1}
so that for interior cells  y[i,j] = W2 @ relu(R_i + C_j) + b2.
Boundary corrections:
    column j=0   uses R0   = W1_a x_i + W1_c x_{i-1} + b1     (drop e-term)
    column j=383 uses R383 = W1_a x_i + W1_e x_{i+1} + b1     (drop c-term)
    row i=0      uses CA   = W1_b x_j + W1_f x_{j-1}          (drop d-term)
    row i=383    uses CB   = W1_b x_j + W1_d x_{j+1}          (drop f-term)
    corners (0,0)/(383,383) need both (handled by patching R0/R383 columns).

Sharding: 8 cores, 48 output rows (i) each, both batches; every core gets the
full (transposed, zero-padded) x so the +-1 shifts are just AP column offsets.
The program is SPMD-uniform; core specialization enters only through per-core
input data (difference stationaries are zero on interior cores).

Schedule (v1): inputs split into 3 parallel DMAs on the SP/ACT/DVE queues so
the transfer is not serialized on one queue; PE warmed with dummy matmuls
during the DMA wait; R-chain (small) runs before the C-chain; main-loop
relu(R_i + C) rows are produced on three engines (DVE tensor_scalar in the
4x perf mode - all operands bf16 SBUF -, ACT activation, Pool tensor_scalar)
and reduced on PE into a [96,N] PSUM accumulator via 32-col zero-slab
stationaries over three rotating column groups; boundary j-columns are
computed mid-loop; the output is extracted + DMA'd in 3 chunks as each
column group's accumulation completes.
"""

import ml_dtypes
import numpy as np
from contextlib import ExitStack

import concourse.bass as bass
import concourse.bacc as bacc
import concourse.tile as tile
from concourse import mybir
from concourse.bass_utils import run_bass_kernel_spmd

B, N, CIN, H = 2, 384, 64, 128
NCORES = 8
RPC = N // NCORES  # rows (i) per core = 48
ROWS = B * RPC     # (b, i) rows per core = 96

F32 = mybir.dt.float32
F32R = mybir.dt.float32r
BF16 = mybir.dt.bfloat16

ADD = mybir.AluOpType.add
MAX = mybir.AluOpType.max
RELU = mybir.ActivationFunctionType.Relu
IDENT = mybir.ActivationFunctionType.Identity

# main-loop row split across the three elementwise engines
N_DVE, N_ACT, N_POOL = 43, 30, 23
assert N_DVE + N_ACT + N_POOL == ROWS

N_WARMUP = 6  # dummy PE matmuls during the input-DMA wait (p-state ramp)

SC_BLOCKS = ["b", "f", "CA3", "dd", "cbdiff"]  # C-chain stationaries
SR_BLOCKS = ["a", "c", "e", "negc"]            # R-chain stationaries


def _engine_sequence():
    """Bresenham-interleave the 96 rows across the three engines."""
    quota = {"v": N_DVE, "s": N_ACT, "p": N_POOL}
    err = {e: 0.0 for e in quota}
    seq = []
    for _ in range(ROWS):
        for e in quota:
            err[e] += quota[e]
        best = max(err, key=lambda e: err[e])
        seq.append(best)
        err[best] -= ROWS
    return seq


def _group_sequence():
    """Column-group visit order: front-load group 0 so its PSUM rows finish
    at k=62 (early extract+store), while never repeating a group
    back-to-back (keeps LDWEIGHTS overlapped with the streaming matmul)."""
    seq = [(0, 1, 0, 2)[k % 4] for k in range(64)]
    seq += [(1, 2)[k % 2] for k in range(32)]
    assert all(seq.count(g) == 32 for g in range(3))
    assert all(seq[i] != seq[i + 1] for i in range(ROWS - 1))
    return seq


def build_program(b2_value: float):
    """Build the SPMD Bass program (same NEFF for all 8 cores)."""
    nc = bacc.Bacc(
        "TRN2", target_bir_lowering=False, debug=False,
        enable_asserts=False, num_devices=NCORES,
    )
    # ---- DRAM I/O: three parallel input streams + two small consts ----
    W_XP = B * (N + 2)
    W_SC = len(SC_BLOCKS) * H
    W_SR = len(SR_BLOCKS) * H + B * (RPC + 2)
    d_xp = nc.dram_tensor("xp", [CIN, W_XP], F32R, kind="ExternalInput").ap()
    d_sc = nc.dram_tensor("sc", [CIN, W_SC], F32R, kind="ExternalInput").ap()
    d_sr = nc.dram_tensor("sr", [CIN, W_SR], F32R, kind="ExternalInput").ap()
    d_b128 = nc.dram_tensor("b128", [H, 64], BF16, kind="ExternalInput").ap()
    d_b1 = nc.dram_tensor("b1c", [H, 2], F32, kind="ExternalInput").ap()
    d_y = nc.dram_tensor("y", [B, RPC, N], F32, kind="ExternalOutput").ap()
    d_yf = d_y.flatten_outer_dims()

    eng_seq = _engine_sequence()
    grp_seq = _group_sequence()
    first_k = {g: grp_seq.index(g) for g in range(3)}
    last_k = {g: ROWS - 1 - grp_seq[::-1].index(g) for g in range(3)}

    with tile.TileContext(nc) as tc, ExitStack() as ctx:
        consts = ctx.enter_context(tc.tile_pool(name="consts", bufs=1))
        cpool = ctx.enter_context(tc.tile_pool(name="cmats", bufs=1))
        rpool = ctx.enter_context(tc.tile_pool(name="rmats", bufs=1))
        mpool = ctx.enter_context(tc.tile_pool(name="mtiles", bufs=9))
        ypool = ctx.enter_context(tc.tile_pool(name="yout", bufs=1))
        ps = ctx.enter_context(tc.tile_pool(name="ps", bufs=4, space="PSUM"))
        yps_pool = ctx.enter_context(
            tc.tile_pool(name="yps", bufs=1, space="PSUM"))

        # ---- input DMAs, one per engine queue so they run in parallel ----
        b1 = consts.tile([H, 2], F32, tag="b1c", name="b1c")
        nc.sync.dma_start(b1[:, :], d_b1)
        b128 = consts.tile([H, 64], BF16, tag="b128", name="b128")
        nc.sync.dma_start(b128[:, :], d_b128)
        xp_t = consts.tile([CIN, W_XP], F32R, tag="xp", name="xp")
        nc.sync.dma_start(xp_t[:, :], d_xp)
        sc_t = consts.tile([CIN, W_SC], F32R, tag="sc", name="sc")
        nc.scalar.dma_start(sc_t[:, :], d_sc)
        sr_t = consts.tile([CIN, W_SR], F32R, tag="sr", name="sr")
        nc.gpsimd.dma_start(sr_t[:, :], d_sr)

        # dependency-free dummy Relu: forces the ACT table-set load to happen
        # during the input-DMA wait instead of later on the critical path
        warm = consts.tile([1, 2], F32, tag="warm", name="warm")
        nc.scalar.activation(warm[0:1, 1:2], warm[0:1, 0:1], RELU,
                             bias=warm[0:1, 0:1])

        # PE p-state warmup: dummy matmuls on a zeroed tile while the input
        # DMAs are in flight (PE clock ramps only while continuously busy)
        wsrc = consts.tile([H, N], BF16, tag="wsrc", name="wsrc")
        nc.gpsimd.memset(wsrc[:, :], 0)
        wps = ps.tile([32, N], F32, tag="ps", name="ps")
        for _ in range(N_WARMUP):
            nc.tensor.matmul(wps[:, :], wsrc[:, 0:32], wsrc[:, :],
                             start=True, stop=True)

        b1col = b1[:, 0:1]
        b2col = b1[:, 1:2]

        SC = {k: sc_t[:, H * i:H * (i + 1)] for i, k in enumerate(SC_BLOCKS)}
        SR = {k: sr_t[:, H * i:H * (i + 1)] for i, k in enumerate(SR_BLOCKS)}
        xsl = [sr_t[:, 4 * H + b * (RPC + 2):4 * H + (b + 1) * (RPC + 2)]
               for b in range(B)]
        xp = [xp_t[:, b * (N + 2):(b + 1) * (N + 2)] for b in range(B)]
        w2slab = b128[:, 0:63]
        w2 = b128[:, 63:64]

        def mmr(out_ap, lhsT_ap, rhs_ap, start, stop):
            nc.tensor.matmul(out_ap, lhsT_ap, rhs_ap, start=start, stop=stop)

        xm = [xp[b][:, 1:N + 1] for b in range(B)]
        xu = [xp[b][:, 0:N] for b in range(B)]
        xd = [xp[b][:, 2:N + 2] for b in range(B)]
        xms = [xsl[b][:, 1:RPC + 1] for b in range(B)]
        xus = [xsl[b][:, 0:RPC] for b in range(B)]
        xds = [xsl[b][:, 2:RPC + 2] for b in range(B)]

        W_ps, R_ps = [], []
        for b in range(B):
            R_ps.append(ps.tile([H, RPC], F32, tag="ps", name="ps"))
        for b in range(B):
            W_ps.append(ps.tile([H, N], F32, tag="ps", name="ps"))

        # ---- R chain first (small matmuls; gates nothing but needs to be
        # ready when the first main-loop rows start) ----
        R_sb, R0_sb, R383_sb = [], [], []
        for b in range(B):
            mmr(R_ps[b][:, :], SR["a"], xms[b], True, False)    # P_a
        for b in range(B):
            mmr(R_ps[b][:, :], SR["c"], xus[b], False, False)   # = R0
        for b in range(B):
            r0 = rpool.tile([H, RPC], F32, tag=f"R0_{b}", name=f"R0_{b}")
            nc.scalar.activation(r0[:, :], R_ps[b][:, :], IDENT, bias=b1col)
            R0_sb.append(r0)
        for b in range(B):
            mmr(R_ps[b][:, :], SR["e"], xds[b], False, False)   # = R
        for b in range(B):
            r = rpool.tile([H, RPC], F32, tag=f"R_{b}", name=f"R_{b}")
            nc.scalar.activation(r[:, :], R_ps[b][:, :], IDENT, bias=b1col)
            R_sb.append(r)
        for b in range(B):
            mmr(R_ps[b][:, :], SR["negc"], xus[b], False, True)  # = R383
        for b in range(B):
            r383 = rpool.tile([H, RPC], F32, tag=f"R383_{b}", name=f"R383_{b}")
            nc.scalar.activation(r383[:, :], R_ps[b][:, :], IDENT, bias=b1col)
            R383_sb.append(r383)

        # ---- C chain: one 5-matmul PSUM accumulation per batch with bf16
        # SBUF snapshots at the CA / C / CB stages (casts split DVE/Pool) ----
        C_sb, CA_sb, CB_sb = [], [], []
        for b in range(B):
            mmr(W_ps[b][:, :], SC["b"], xm[b], True, False)     # P_b
        for b in range(B):
            mmr(W_ps[b][:, :], SC["f"], xu[b], False, False)    # + P_f
        for b in range(B):
            mmr(W_ps[b][:, :], SC["CA3"], xd[b], False, False)  # = CA
        for b in range(B):
            ca = cpool.tile([H, N], BF16, tag=f"CA_{b}", name=f"CA_{b}")
            eng = nc.vector if b == 0 else nc.scalar
            if b == 0:
                eng.tensor_copy(ca[:, :], W_ps[b][:, :])
            else:
                eng.activation(ca[:, :], W_ps[b][:, :], IDENT)
            CA_sb.append(ca)
        for b in range(B):
            mmr(W_ps[b][:, :], SC["dd"], xd[b], False, False)   # = C
        for b in range(B):
            c = cpool.tile([H, N], BF16, tag=f"C_{b}", name=f"C_{b}")
            if b == 0:
                nc.vector.tensor_copy(c[:, :], W_ps[b][:, :])
            else:
                nc.scalar.activation(c[:, :], W_ps[b][:, :], IDENT)
            C_sb.append(c)
        for b in range(B):
            mmr(W_ps[b][:, :], SC["cbdiff"], xu[b], False, True)  # = CB
        # CB casts are emitted mid-loop (DVE) right before their first use
        for b in range(B):
            cb = cpool.tile([H, N], BF16, tag=f"CB_{b}", name=f"CB_{b}")
            CB_sb.append(cb)

        # ---- output staging ----
        Y = ypool.tile([ROWS, N], F32, tag="Y", name="Y")
        yacc = yps_pool.tile([H, N], F32, tag="yacc", name="yacc")

        def extract_group(g):
            """PSUM -> Y for column group g (interior columns), fold +b2."""
            rs = slice(32 * g, 32 * g + 32)
            if g == 2:
                nc.scalar.activation(Y[rs, 1:N - 1], yacc[rs, 1:N - 1],
                                     IDENT, bias=b2col[rs, :])
            else:
                nc.vector.tensor_scalar_add(Y[rs, 1:N - 1], yacc[rs, 1:N - 1],
                                            b2_value)
            nc.sync.dma_start(d_yf[rs, 1:N - 1], Y[rs, 1:N - 1])

        def boundary_block():
            """Corner patches + boundary columns j=0 / j=383 (mid-loop, on
            Pool/ACT slack; PE cost is 2 small matmuls)."""
            yc_ps = ps.tile([ROWS, 2], F32, tag="ps", name="ps")
            for b in range(B):
                dca = rpool.tile([H, 1], F32, tag=f"dca_{b}", name=f"dca_{b}")
                nc.gpsimd.tensor_sub(dca[:, :], CA_sb[b][:, 0:1],
                                     C_sb[b][:, 0:1])
                nc.gpsimd.tensor_add(R0_sb[b][:, 0:1], R0_sb[b][:, 0:1],
                                     dca[:, :])
                dcb = rpool.tile([H, 1], F32, tag=f"dcb_{b}", name=f"dcb_{b}")
                nc.gpsimd.tensor_sub(dcb[:, :], CB_sb[b][:, N - 1:N],
                                     C_sb[b][:, N - 1:N])
                nc.gpsimd.tensor_add(R383_sb[b][:, RPC - 1:RPC],
                                     R383_sb[b][:, RPC - 1:RPC], dcb[:, :])
            for ci, col in enumerate((0, N - 1)):
                mc = mpool.tile([H, ROWS], BF16, tag="m", name="m")
                for b in range(B):
                    rt = R0_sb[b] if col == 0 else R383_sb[b]
                    csc = rpool.tile([H, 1], F32, tag=f"csc{ci}_{b}",
                                     name=f"csc{ci}_{b}")
                    nc.gpsimd.tensor_copy(csc[:, :], C_sb[b][:, col:col + 1])
                    nc.gpsimd.tensor_scalar(mc[:, b * RPC:(b + 1) * RPC],
                                            rt[:, :], csc[:, :], 0.0, ADD, MAX)
                nc.tensor.matmul(yc_ps[:, ci:ci + 1], mc[:, :], w2,
                                 start=True, stop=True)
                nc.scalar.activation(Y[:, col:col + 1], yc_ps[:, ci:ci + 1],
                                     IDENT, bias=b2col[0:ROWS, :])
            nc.sync.dma_start(d_yf[:, 0:1], Y[:, 0:1])
            nc.sync.dma_start(d_yf[:, N - 1:N], Y[:, N - 1:N])

        # ---- main loop: 96 rows; group-g rows land on PSUM partitions
        # 32g..32g+31; groups extracted + stored as they complete ----
        cnt = {0: 0, 1: 0, 2: 0}
        for k in range(ROWS):
            g = grp_seq[k]
            p = 32 * g + cnt[g]
            cnt[g] += 1
            b, i = divmod(p, RPC)
            if i == 0:
                cin = CA_sb[b]
            elif i == RPC - 1:
                cin = CB_sb[b]
            else:
                cin = C_sb[b]
            m = mpool.tile([H, N], BF16, tag="m", name="m")
            rcol = R_sb[b][:, i:i + 1]
            if eng_seq[k] == "v":
                nc.vector.tensor_scalar(m[:, :], cin[:, :], rcol, 0.0, ADD, MAX)
            elif eng_seq[k] == "s":
                nc.scalar.activation(m[:, :], cin[:, :], RELU, bias=rcol)
            else:
                nc.gpsimd.tensor_scalar(m[:, :], cin[:, :], rcol, 0.0, ADD, MAX)
            col = p % 32
            stat = w2slab[:, 31 - col: 63 - col]
            nc.tensor.matmul(yacc[32 * g:32 * g + 32, :], stat, m[:, :],
                             start=(k == first_k[g]), stop=(k == last_k[g]),
                             tile_position=(0, 32 * g))
            if k == 20:
                # CB snapshots (PSUM -> SBUF bf16), first needed at k=46
                for b in range(B):
                    nc.vector.tensor_copy(CB_sb[b][:, :], W_ps[b][:, :])
            if k == last_k[g]:
                extract_group(g)
            if k == 70:
                boundary_block()

    nc.compile()
    return nc


def _prep_inputs(x, W1, b1, W2, b2):
    """Host-side restructuring (layout only, no FLOPs beyond tiny S diffs)."""
    x = np.asarray(x, np.float32)
    W1 = np.asarray(W1, np.float32)
    b1 = np.asarray(b1, np.float32)
    W2 = np.asarray(W2, np.float32)
    b2v = float(np.asarray(b2).reshape(-1)[0])
    xp = np.zeros((B, CIN, N + 2), np.float32)
    xp[:, :, 1:N + 1] = x.transpose(0, 2, 1)
    S = {k: np.ascontiguousarray(W1[:, 64 * i:64 * (i + 1)].T)
         for i, k in enumerate("abcdef")}
    w2slab = np.zeros((H, 63), np.float32)
    w2slab[:, 31] = W2.reshape(H)
    b128 = np.concatenate([w2slab, W2.reshape(1, H).T], axis=1)
    b128 = b128.astype(ml_dtypes.bfloat16)
    b1c = np.concatenate([b1.reshape(H, 1),
                          np.full((H, 1), b2v, np.float32)], axis=1)
    return xp, S, b128, b1c, b2v


def kernel(x, W1, b1, W2, b2, trace=False):
    xp, S, b128, b1c, b2v = _prep_inputs(x, W1, b1, W2, b2)
    nc = build_program(b2v)

    zeros_s = np.zeros((CIN, H), np.float32)
    d_xp = np.ascontiguousarray(np.concatenate([xp[0], xp[1]], axis=1))
    in_maps = []
    for c in range(NCORES):
        lo = c * RPC
        xsl = [xp[b, :, lo:lo + RPC + 2] for b in range(B)]
        sca3 = zeros_s if c == 0 else S["d"]
        scb3 = zeros_s if c == NCORES - 1 else S["f"]
        blocks = {
            "a": S["a"], "b": S["b"], "c": S["c"], "e": S["e"], "f": S["f"],
            "CA3": sca3,
            "dd": S["d"] - sca3,
            "cbdiff": scb3 - S["f"],
            "negc": -S["c"],
        }
        d_sc = np.concatenate([blocks[k] for k in SC_BLOCKS], axis=1)
        d_sr = np.concatenate([blocks[k] for k in SR_BLOCKS] + xsl, axis=1)
        in_maps.append({
            "xp": d_xp,
            "sc": np.ascontiguousarray(d_sc),
            "sr": np.ascontiguousarray(d_sr),
            "b128": b128,
            "b1c": b1c,
        })

    res = run_bass_kernel_spmd(nc, in_maps, core_ids=list(range(NCORES)),
                               trace=trace)
    y = np.concatenate([res.results[c]["y"] for c in range(NCORES)], axis=1)
    y = y.reshape(B, N, N, 1).astype(np.float32)
    if trace:
        return y, res
    return y


# revision 10
# speedup vs baseline: 3.8386x; 3.8386x over previous
"""Trainium2 Bass kernel for nn_FCPairedLayer (gnn_message_passing).

Reference computation:
    v[b,i,j] = concat(x_i, x_j, x_{i-1}*m1, x_{j+1}*m1, x_{i+1}*m2, x_{j-1}*m2)
    y[b,i,j] = W2 @ relu(W1 @ v + b1) + b2        (scalar output per pair)
with m1 = [i>=1][j<=N-2], m2 = [i<=N-2][j>=1].

W1 @ v splits into row-only and column-only terms; per batch define
    R[:,i] = W1_a x_i + W1_c x_{i-1} + W1_e x_{i+1} + b1     (shifts masked)
    C[:,j] = W1_b x_j + W1_d x_{j+1} + W1_f x_{j-1}
so that for interior cells  y[i,j] = W2 @ relu(R_i + C_j) + b2.
Boundary corrections:
    column j=0   uses R0   = W1_a x_i + W1_c x_{i-1} + b1     (drop e-term)
    column j=383 uses R383 = W1_a x_i + W1_e x_{i+1} + b1     (drop c-term)
    row i=0      uses CA   = W1_b x_j + W1_f x_{j-1}          (drop d-term)
    row i=383    uses CB   = W1_b x_j + W1_d x_{j+1}          (drop f-term)
    corners (0,0)/(383,383) need both (handled by patching R0/R383 columns).

Sharding: 8 cores, 48 output rows (i) each, both batches; every core gets the
full (transposed, zero-padded) x so the +-1 shifts are just AP column offsets.
The program is SPMD-uniform; core specialization enters only through per-core
input data (difference stationaries are zero on interior cores).

Schedule (v1): inputs split into 3 parallel DMAs on the SP/ACT/DVE queues so
the transfer is not serialized on one queue; PE warmed with dummy matmuls
during the DMA wait; R-chain (small) runs before the C-chain; main-loop
relu(R_i + C) rows are produced on three engines (DVE tensor_scalar in the
4x perf mode - all operands bf16 SBUF -, ACT activation, Pool tensor_scalar)
and reduced on PE into a [96,N] PSUM accumulator via 32-col zero-slab
stationaries over three rotating column groups; boundary j-columns are
computed mid-loop; the output is extracted + DMA'd in 3 chunks as each
column group's accumulation completes.
"""

import ml_dtypes
import numpy as np
from contextlib import ExitStack

import concourse.bass as bass
import concourse.bacc as bacc
import concourse.tile as tile
from concourse import mybir
from concourse.bass_utils import run_bass_kernel_spmd

B, N, CIN, H = 2, 384, 64, 128
NCORES = 8
RPC = N // NCORES  # rows (i) per core = 48
ROWS = B * RPC     # (b, i) rows per core = 96

F32 = mybir.dt.float32
F32R = mybir.dt.float32r
BF16 = mybir.dt.bfloat16

ADD = mybir.AluOpType.add
MAX = mybir.AluOpType.max
RELU = mybir.ActivationFunctionType.Relu
IDENT = mybir.ActivationFunctionType.Identity

# main-loop row split across the three elementwise engines
N_DVE, N_ACT, N_POOL = 58, 38, 0
assert N_DVE + N_ACT + N_POOL == ROWS

N_WARMUP = 6  # dummy PE matmuls during the input-DMA wait (p-state ramp)

SC_BLOCKS = ["b", "f", "CA3", "dd", "cbdiff"]  # C-chain stationaries
SR_BLOCKS = ["a", "c", "e", "negc"]            # R-chain stationaries


def _engine_sequence():
    """Bresenham-interleave the 96 rows across the three engines."""
    quota = {"v": N_DVE, "s": N_ACT, "p": N_POOL}
    err = {e: 0.0 for e in quota}
    seq = []
    for _ in range(ROWS):
        for e in quota:
            err[e] += quota[e]
        best = max(err, key=lambda e: err[e])
        seq.append(best)
        err[best] -= ROWS
    return seq


def _group_sequence():
    """Column-group visit order: front-load group 0 so its PSUM rows finish
    at k=62 (early extract+store), while never repeating a group
    back-to-back (keeps LDWEIGHTS overlapped with the streaming matmul)."""
    seq = [(0, 1, 0, 2)[k % 4] for k in range(64)]
    seq += [(1, 2)[k % 2] for k in range(32)]
    assert all(seq.count(g) == 32 for g in range(3))
    assert all(seq[i] != seq[i + 1] for i in range(ROWS - 1))
    return seq


def build_program(b2_value: float):
    """Build the SPMD Bass program (same NEFF for all 8 cores)."""
    nc = bacc.Bacc(
        "TRN2", target_bir_lowering=False, debug=False,
        enable_asserts=False, num_devices=NCORES,
    )
    # ---- DRAM I/O: three parallel input streams + two small consts ----
    W_XP = B * (N + 2)
    W_SC = len(SC_BLOCKS) * H
    W_SR = len(SR_BLOCKS) * H + B * (RPC + 2)
    d_xp = nc.dram_tensor("xp", [CIN, W_XP], F32R, kind="ExternalInput").ap()
    d_sc = nc.dram_tensor("sc", [CIN, W_SC], F32R, kind="ExternalInput").ap()
    d_sr = nc.dram_tensor("sr", [CIN, W_SR], F32R, kind="ExternalInput").ap()
    d_b128 = nc.dram_tensor("b128", [H, 64], BF16, kind="ExternalInput").ap()
    d_b1 = nc.dram_tensor("b1c", [H, 2], F32, kind="ExternalInput").ap()
    d_y = nc.dram_tensor("y", [B, RPC, N], F32, kind="ExternalOutput").ap()
    d_yf = d_y.flatten_outer_dims()

    eng_seq = _engine_sequence()
    grp_seq = _group_sequence()
    first_k = {g: grp_seq.index(g) for g in range(3)}
    last_k = {g: ROWS - 1 - grp_seq[::-1].index(g) for g in range(3)}

    with tile.TileContext(nc) as tc, ExitStack() as ctx:
        consts = ctx.enter_context(tc.tile_pool(name="consts", bufs=1))
        cpool = ctx.enter_context(tc.tile_pool(name="cmats", bufs=1))
        rpool = ctx.enter_context(tc.tile_pool(name="rmats", bufs=1))
        mpool = ctx.enter_context(tc.tile_pool(name="mtiles", bufs=9))
        ypool = ctx.enter_context(tc.tile_pool(name="yout", bufs=1))
        ps = ctx.enter_context(tc.tile_pool(name="ps", bufs=4, space="PSUM"))
        yps_pool = ctx.enter_context(
            tc.tile_pool(name="yps", bufs=1, space="PSUM"))

        # ---- input DMAs, one per engine queue so they run in parallel;
        # sr (R-chain stationaries, first consumer) on the fast SP queue ----
        sr_t = consts.tile([CIN, W_SR], F32R, tag="sr", name="sr")
        nc.sync.dma_start(sr_t[:, :], d_sr)
        b1 = consts.tile([H, 2], F32, tag="b1c", name="b1c")
        nc.sync.dma_start(b1[:, :], d_b1)
        b128 = consts.tile([H, 64], BF16, tag="b128", name="b128")
        nc.sync.dma_start(b128[:, :], d_b128)
        sc_t = consts.tile([CIN, W_SC], F32R, tag="sc", name="sc")
        nc.scalar.dma_start(sc_t[:, :], d_sc)
        xp_t = consts.tile([CIN, W_XP], F32R, tag="xp", name="xp")
        nc.gpsimd.dma_start(xp_t[:, :], d_xp)

        # dependency-free dummy Relu: forces the ACT table-set load to happen
        # during the input-DMA wait instead of later on the critical path
        warm = consts.tile([1, 2], F32, tag="warm", name="warm")
        nc.scalar.activation(warm[0:1, 1:2], warm[0:1, 0:1], RELU,
                             bias=warm[0:1, 0:1])

        # PE p-state warmup: dummy matmuls on a zeroed tile while the input
        # DMAs are in flight (PE clock ramps only while continuously busy)
        wsrc = consts.tile([H, N], BF16, tag="wsrc", name="wsrc")
        nc.gpsimd.memset(wsrc[:, :], 0)
        wprb = consts.tile([H, N], BF16, tag="wprb", name="wprb")
        # throughput probes (results unused): Pool tensor_tensor add with
        # broadcast in1; DVE STT and TT-broadcast forms
        nc.gpsimd.tensor_add(wprb[:, :], wsrc[:, :],
                             wsrc[:, 0:1].broadcast_to([H, N]))
        nc.vector.scalar_tensor_tensor(wprb[:, :], wsrc[:, :], 0.0,
                                       wsrc[:, :], ADD, MAX)
        nc.vector.tensor_max(wprb[:, :], wsrc[:, :],
                             wsrc[:, 0:1].broadcast_to([H, N]))
        nc.vector.tensor_scalar(wprb[:, :], wsrc[:, :], 0.0, None, MAX)
        wps = ps.tile([32, N], F32, tag="ps", name="ps")
        for _ in range(N_WARMUP):
            nc.tensor.matmul(wps[:, :], wsrc[:, 0:32], wsrc[:, :],
                             start=True, stop=True)

        b1col = b1[:, 0:1]
        b2col = b1[:, 1:2]

        SC = {k: sc_t[:, H * i:H * (i + 1)] for i, k in enumerate(SC_BLOCKS)}
        SR = {k: sr_t[:, H * i:H * (i + 1)] for i, k in enumerate(SR_BLOCKS)}
        xsl = [sr_t[:, 4 * H + b * (RPC + 2):4 * H + (b + 1) * (RPC + 2)]
               for b in range(B)]
        xp = [xp_t[:, b * (N + 2):(b + 1) * (N + 2)] for b in range(B)]
        w2slab = b128[:, 0:63]
        w2 = b128[:, 63:64]

        def mmr(out_ap, lhsT_ap, rhs_ap, start, stop):
            nc.tensor.matmul(out_ap, lhsT_ap, rhs_ap, start=start, stop=stop)

        xm = [xp[b][:, 1:N + 1] for b in range(B)]
        xu = [xp[b][:, 0:N] for b in range(B)]
        xd = [xp[b][:, 2:N + 2] for b in range(B)]
        xms = [xsl[b][:, 1:RPC + 1] for b in range(B)]
        xus = [xsl[b][:, 0:RPC] for b in range(B)]
        xds = [xsl[b][:, 2:RPC + 2] for b in range(B)]

        W_ps, R_ps = [], []
        for b in range(B):
            R_ps.append(ps.tile([H, RPC], F32, tag="ps", name="ps"))
        for b in range(B):
            W_ps.append(ps.tile([H, N], F32, tag="ps", name="ps"))

        # ---- R chain first (small matmuls; gates nothing but needs to be
        # ready when the first main-loop rows start) ----
        R_sb, R0_sb, R383_sb = [], [], []
        for b in range(B):
            mmr(R_ps[b][:, :], SR["a"], xms[b], True, False)    # P_a
        for b in range(B):
            mmr(R_ps[b][:, :], SR["c"], xus[b], False, False)   # = R0
        for b in range(B):
            r0 = rpool.tile([H, RPC], F32, tag=f"R0_{b}", name=f"R0_{b}")
            nc.scalar.activation(r0[:, :], R_ps[b][:, :], IDENT, bias=b1col)
            R0_sb.append(r0)
        for b in range(B):
            mmr(R_ps[b][:, :], SR["e"], xds[b], False, False)   # = R
        for b in range(B):
            r = rpool.tile([H, RPC], F32, tag=f"R_{b}", name=f"R_{b}")
            nc.scalar.activation(r[:, :], R_ps[b][:, :], IDENT, bias=b1col)
            R_sb.append(r)
        for b in range(B):
            mmr(R_ps[b][:, :], SR["negc"], xus[b], False, True)  # = R383
        for b in range(B):
            r383 = rpool.tile([H, RPC], F32, tag=f"R383_{b}", name=f"R383_{b}")
            nc.scalar.activation(r383[:, :], R_ps[b][:, :], IDENT, bias=b1col)
            R383_sb.append(r383)

        # ---- C chain: one 5-matmul PSUM accumulation per batch with bf16
        # SBUF snapshots at the CA / C / CB stages (casts split DVE/Pool) ----
        C_sb, CA_sb, CB_sb = [], [], []
        for b in range(B):
            mmr(W_ps[b][:, :], SC["b"], xm[b], True, False)     # P_b
        for b in range(B):
            mmr(W_ps[b][:, :], SC["f"], xu[b], False, False)    # + P_f
        for b in range(B):
            mmr(W_ps[b][:, :], SC["CA3"], xd[b], False, False)  # = CA
        for b in range(B):
            ca = cpool.tile([H, N], BF16, tag=f"CA_{b}", name=f"CA_{b}")
            eng = nc.vector if b == 0 else nc.scalar
            if b == 0:
                eng.tensor_copy(ca[:, :], W_ps[b][:, :])
            else:
                eng.activation(ca[:, :], W_ps[b][:, :], IDENT)
            CA_sb.append(ca)
        for b in range(B):
            mmr(W_ps[b][:, :], SC["dd"], xd[b], False, False)   # = C
        for b in range(B):
            c = cpool.tile([H, N], BF16, tag=f"C_{b}", name=f"C_{b}")
            if b == 0:
                nc.vector.tensor_copy(c[:, :], W_ps[b][:, :])
            else:
                nc.scalar.activation(c[:, :], W_ps[b][:, :], IDENT)
            C_sb.append(c)
        for b in range(B):
            mmr(W_ps[b][:, :], SC["cbdiff"], xu[b], False, True)  # = CB
        # CB casts are emitted mid-loop (DVE) right before their first use
        for b in range(B):
            cb = cpool.tile([H, N], BF16, tag=f"CB_{b}", name=f"CB_{b}")
            CB_sb.append(cb)

        # ---- output staging ----
        Y = ypool.tile([ROWS, N], F32, tag="Y", name="Y")
        yacc = yps_pool.tile([H, N], F32, tag="yacc", name="yacc")

        def extract_group(g):
            """PSUM -> Y for column group g (interior columns), fold +b2."""
            rs = slice(32 * g, 32 * g + 32)
            if g == 2:
                nc.scalar.activation(Y[rs, 1:N - 1], yacc[rs, 1:N - 1],
                                     IDENT, bias=b2col[rs, :])
            else:
                nc.vector.tensor_scalar_add(Y[rs, 1:N - 1], yacc[rs, 1:N - 1],
                                            b2_value)
            nc.sync.dma_start(d_yf[rs, 1:N - 1], Y[rs, 1:N - 1])

        def boundary_block():
            """Corner patches + boundary columns j=0 / j=383 (mid-loop, on
            Pool/ACT slack; PE cost is 2 small matmuls)."""
            yc_ps = ps.tile([ROWS, 2], F32, tag="ps", name="ps")
            for b in range(B):
                dca = rpool.tile([H, 1], F32, tag=f"dca_{b}", name=f"dca_{b}")
                nc.gpsimd.tensor_sub(dca[:, :], CA_sb[b][:, 0:1],
                                     C_sb[b][:, 0:1])
                nc.gpsimd.tensor_add(R0_sb[b][:, 0:1], R0_sb[b][:, 0:1],
                                     dca[:, :])
                dcb = rpool.tile([H, 1], F32, tag=f"dcb_{b}", name=f"dcb_{b}")
                nc.gpsimd.tensor_sub(dcb[:, :], CB_sb[b][:, N - 1:N],
                                     C_sb[b][:, N - 1:N])
                nc.gpsimd.tensor_add(R383_sb[b][:, RPC - 1:RPC],
                                     R383_sb[b][:, RPC - 1:RPC], dcb[:, :])
            for ci, col in enumerate((0, N - 1)):
                mc = mpool.tile([H, ROWS], BF16, tag="m", name="m")
                for b in range(B):
                    rt = R0_sb[b] if col == 0 else R383_sb[b]
                    csc = rpool.tile([H, 1], F32, tag=f"csc{ci}_{b}",
                                     name=f"csc{ci}_{b}")
                    nc.gpsimd.tensor_copy(csc[:, :], C_sb[b][:, col:col + 1])
                    nc.vector.tensor_scalar(mc[:, b * RPC:(b + 1) * RPC],
                                            rt[:, :], csc[:, :], 0.0, ADD, MAX)
                nc.tensor.matmul(yc_ps[:, ci:ci + 1], mc[:, :], w2,
                                 start=True, stop=True)
                nc.scalar.activation(Y[:, col:col + 1], yc_ps[:, ci:ci + 1],
                                     IDENT, bias=b2col[0:ROWS, :])
            nc.sync.dma_start(d_yf[:, 0:1], Y[:, 0:1])
            nc.sync.dma_start(d_yf[:, N - 1:N], Y[:, N - 1:N])

        # ---- main loop: 96 rows; group-g rows land on PSUM partitions
        # 32g..32g+31; groups extracted + stored as they complete ----
        cnt = {0: 0, 1: 0, 2: 0}
        for k in range(ROWS):
            g = grp_seq[k]
            p = 32 * g + cnt[g]
            cnt[g] += 1
            b, i = divmod(p, RPC)
            if i == 0:
                cin = CA_sb[b]
            elif i == RPC - 1:
                cin = CB_sb[b]
            else:
                cin = C_sb[b]
            m = mpool.tile([H, N], BF16, tag="m", name="m")
            rcol = R_sb[b][:, i:i + 1]
            if eng_seq[k] == "v":
                nc.vector.tensor_scalar(m[:, :], cin[:, :], rcol, 0.0, ADD, MAX)
            elif eng_seq[k] == "s":
                nc.scalar.activation(m[:, :], cin[:, :], RELU, bias=rcol)
            else:
                nc.gpsimd.tensor_scalar(m[:, :], cin[:, :], rcol, 0.0, ADD, MAX)
            col = p % 32
            stat = w2slab[:, 31 - col: 63 - col]
            nc.tensor.matmul(yacc[32 * g:32 * g + 32, :], stat, m[:, :],
                             start=(k == first_k[g]), stop=(k == last_k[g]),
                             tile_position=(0, 32 * g))
            if k == 20:
                # CB snapshots (PSUM -> SBUF bf16), first needed at k=46
                for b in range(B):
                    nc.vector.tensor_copy(CB_sb[b][:, :], W_ps[b][:, :])
            if k == last_k[g]:
                extract_group(g)
            if k == 70:
                boundary_block()

    nc.compile()
    return nc


def _prep_inputs(x, W1, b1, W2, b2):
    """Host-side restructuring (layout only, no FLOPs beyond tiny S diffs)."""
    x = np.asarray(x, np.float32)
    W1 = np.asarray(W1, np.float32)
    b1 = np.asarray(b1, np.float32)
    W2 = np.asarray(W2, np.float32)
    b2v = float(np.asarray(b2).reshape(-1)[0])
    xp = np.zeros((B, CIN, N + 2), np.float32)
    xp[:, :, 1:N + 1] = x.transpose(0, 2, 1)
    S = {k: np.ascontiguousarray(W1[:, 64 * i:64 * (i + 1)].T)
         for i, k in enumerate("abcdef")}
    w2slab = np.zeros((H, 63), np.float32)
    w2slab[:, 31] = W2.reshape(H)
    b128 = np.concatenate([w2slab, W2.reshape(1, H).T], axis=1)
    b128 = b128.astype(ml_dtypes.bfloat16)
    b1c = np.concatenate([b1.reshape(H, 1),
                          np.full((H, 1), b2v, np.float32)], axis=1)
    return xp, S, b128, b1c, b2v


def kernel(x, W1, b1, W2, b2, trace=False):
    xp, S, b128, b1c, b2v = _prep_inputs(x, W1, b1, W2, b2)
    nc = build_program(b2v)

    zeros_s = np.zeros((CIN, H), np.float32)
    d_xp = np.ascontiguousarray(np.concatenate([xp[0], xp[1]], axis=1))
    in_maps = []
    for c in range(NCORES):
        lo = c * RPC
        xsl = [xp[b, :, lo:lo + RPC + 2] for b in range(B)]
        sca3 = zeros_s if c == 0 else S["d"]
        scb3 = zeros_s if c == NCORES - 1 else S["f"]
        blocks = {
            "a": S["a"], "b": S["b"], "c": S["c"], "e": S["e"], "f": S["f"],
            "CA3": sca3,
            "dd": S["d"] - sca3,
            "cbdiff": scb3 - S["f"],
            "negc": -S["c"],
        }
        d_sc = np.concatenate([blocks[k] for k in SC_BLOCKS], axis=1)
        d_sr = np.concatenate([blocks[k] for k in SR_BLOCKS] + xsl, axis=1)
        in_maps.append({
            "xp": d_xp,
            "sc": np.ascontiguousarray(d_sc),
            "sr": np.ascontiguousarray(d_sr),
            "b128": b128,
            "b1c": b1c,
        })

    res = run_bass_kernel_spmd(nc, in_maps, core_ids=list(range(NCORES)),
                               trace=trace)
    y = np.concatenate([res.results[c]["y"] for c in range(NCORES)], axis=1)
    y = y.reshape(B, N, N, 1).astype(np.float32)
    if trace:
        return y, res
    return y
